# revision 12
# baseline (speedup 1.0000x reference)
"""LocallyConnected2d Trainium2 kernel (bf16 pipeline).

Problem: out[b,o,oh,ow] = sum_{c,ki,kj} x[b,c,oh+ki,ow+kj] * W[o,oh,ow,c,ki,kj] + bias[o,oh,ow]
Shapes: x[32,32,64,64], W[64,62,62,32,3,3], bias[64,62,62] -> out[32,64,62,62], fp32 I/O.

The untied weight tensor (283 MB fp32) is read exactly once -> the kernel is
HBM-bandwidth bound. All operands ship as bf16 (accuracy gate 2e-2 vs ~2e-3
bf16 quantization error), halving the dominant weight stream; PSUM accumulates
in fp32; the output returns as bf16 and is upcast on host.

Strategy (8 NeuronCores, sharded over output rows, 8 rows/core padded to 64):
- Per output location: 3 accumulating PE matmuls, K=97 each (chunk q = kernel
  row ki; features j=(kj,c) plus a ones-row at j=96 that carries bias on q=2).
- lhsT (stationary) = x patch columns [97,32b]: x is loaded into SBUF once as
  3 column-shifted replicas on partitions kj*32+c, so every lhsT is a direct
  AP slice (no im2col data movement). Partition 96 = constant 1.0.
- rhs (moving) = per-location weights [97,64o], streamed from HBM in
  half-row strips with a host-side layout [row, half, j, q, ow, o] making each
  strip one fully-contiguous DMA (97 x 11.9KB descriptors).
- One PSUM bank [128,512] per strip accumulates 8 location-groups (4 locations
  x 32b on partitions, 64o per group in free); a single DVE copy casts the
  bank to a bf16 SBUF strip; one contiguous DMA per half-row out.
"""

import numpy as np
import ml_dtypes

import concourse.bass as bass  # noqa: F401
import concourse.mybir as mybir
import concourse.tile as tile
from concourse import bacc
from concourse.bass_utils import run_bass_kernel_spmd

B, C_IN, H, W = 32, 32, 64, 64
C_OUT, OH, OW, KK = 64, 62, 62, 3
N_CORES = 8
ROWS = 8          # padded output rows per core (8*8=64 >= 62)
HALF = 31         # locations per strip (half an output row)
XH = ROWS + 2     # input rows needed per core
KP = 97           # contraction per chunk: 96 features + ones/bias row
NG = 8            # ceil(31/4) location groups per strip
F32 = mybir.dt.float32
BF16 = mybir.dt.bfloat16
NP_BF16 = ml_dtypes.bfloat16

_NC_CACHE = {}


def _build_nc():
    nc = bacc.Bacc(
        "TRN2",
        target_bir_lowering=False,
        debug=False,
        enable_asserts=False,
        num_devices=N_CORES,
    )
    # x ships host-transposed AND pre-shifted into 3 kj-replicas
    # [kj, c, h, w(62), b] so the whole x3 load is one contiguous DMA
    x_d = nc.dram_tensor("x", [KK, C_IN, XH, OW, B], BF16, kind="ExternalInput").ap()
    # w ships pre-split by half-row strip: [row, half, j, (q l o)+pad]. The
    # 32-elem line pad makes the DRAM source non-contiguous across
    # partitions: a fully-contiguous source lets the HWDGE M2S-concat fuse
    # all descriptors into ONE SDMA engine's stream (~27 GB/s measured);
    # with stride != length the 97 line-descriptors spread over all 16
    # engines (~16x).
    WLINE = 3 * HALF * C_OUT + 32  # 5984
    w_d = nc.dram_tensor(
        "w", [ROWS, 2, KP, WLINE], BF16, kind="ExternalInput"
    ).ap()
    ones_d = nc.dram_tensor("ones", [1, XH * OW * B], BF16, kind="ExternalInput").ap()
    # out layout: [p=(l4,b), strip, grp, o] - partition-major so each store
    # DMA covers several strips with fat contiguous per-partition lines;
    # host unscrambles + upcasts
    o_d = nc.dram_tensor(
        "out", [128, ROWS * 2 * NG * C_OUT], BF16, kind="ExternalOutput"
    ).ap()

    with tile.TileContext(nc) as tc:
        with (
            tc.tile_pool(name="xpool", bufs=1) as xpool,
            tc.tile_pool(name="wpool", bufs=6) as wpool,
            tc.tile_pool(name="opool", bufs=1) as opool,
            tc.tile_pool(name="pspool", bufs=3, space="PSUM") as pspool,
        ):
            # x replicas: partition kj*32+c holds x[b,c,h,w+kj] at free
            # (h, w, b); partition 96 = 1.0 (carries the bias row).
            # SWDGE (gpsimd) sprays each partition line into 16 tiny
            # descriptors (~97 GB/s measured); everything rides the two
            # HWDGE rings (sync=weights, scalar=x tail + out) instead.
            HZ = OW * B  # 1984
            x3 = xpool.tile([KP, XH * HZ], BF16)
            xsrc = x_d.rearrange("k c h w b -> (k c) (h w b)")

            def load_x_rows(r0, r1, eng):
                eng.dma_start(
                    out=x3[0:96, r0 * HZ : r1 * HZ],
                    in_=xsrc[0:96, r0 * HZ : r1 * HZ],
                )

            # rows 0-2 feed strip 0; they go ahead of the w strips on the
            # sync ring. Rows 3-9 + ones ride the scalar ring concurrently.
            load_x_rows(0, 3, nc.sync)
            nc.scalar.dma_start(out=x3[96:97, :], in_=ones_d)
            load_x_rows(3, 6, nc.scalar)
            load_x_rows(6, 10, nc.scalar)

            QZ = HALF * C_OUT  # 1984, one chunk per kernel row q
            ot = opool.tile([128, ROWS * 2 * NG * C_OUT], BF16)  # all strips
            for row in range(ROWS):
                for half in range(2):
                    strip = row * 2 + half
                    # +32 elem pitch pad: a flat [97, 5952] SBUF dest AP
                    # collapses the whole transfer onto one SDMA engine;
                    # pitch != length keeps the 97 line-descriptors spread
                    # across all 16 engines
                    wt_full = wpool.tile([KP, 3 * QZ + 32], BF16, tag="wt")
                    wt = wt_full[:, 0 : 3 * QZ]
                    wsrc = w_d[row, half][:, 0 : 3 * QZ]
                    if strip in (0, 15):
                        # split by q-chunk so the first q=0 matmuls of the
                        # first/last strip unblock after 1/3 of the strip
                        for f0, f1 in ((0, QZ), (QZ, 2 * QZ), (2 * QZ, 3 * QZ)):
                            nc.sync.dma_start(
                                out=wt[:, f0:f1], in_=wsrc[:, f0:f1]
                            )
                    else:
                        nc.sync.dma_start(out=wt, in_=wsrc)
                    # one PSUM bank for the whole strip: partitions (l4,b),
                    # free (grp, o)
                    ps = pspool.tile([128, NG * C_OUT], F32, tag="ps")
                    for g in range(NG):
                        gn = min(4, HALF - g * 4)  # 4,4,...,3
                        for li in range(4):
                            # pad slot in the last group duplicates the prior
                            # location (keeps PSUM fully written; host drops it)
                            eff = min(li, gn - 1)
                            ow = half * HALF + g * 4 + eff
                            loff = (g * 4 + eff) * C_OUT
                            for q in range(3):
                                nc.tensor.matmul(
                                    ps[32 * li : 32 * li + 32, g * C_OUT : (g + 1) * C_OUT],
                                    x3[
                                        :,
                                        (row + q) * HZ
                                        + ow * B : (row + q) * HZ
                                        + ow * B
                                        + B,
                                    ],  # [97, 32] lhsT
                                    wt[:, q * QZ + loff : q * QZ + loff + C_OUT],
                                    start=(q == 0),
                                    stop=(q == 2),
                                    tile_position=(0, 32 * li),
                                )
                    SZ = NG * C_OUT  # 512 out elems per strip per partition
                    nc.vector.tensor_copy(
                        out=ot[:, strip * SZ : (strip + 1) * SZ], in_=ps
                    )
                    # store in 4-strip chunks: 4KB contiguous per-partition
                    # lines on the scalar HWDGE ring
                    if strip % 4 == 3:
                        c0 = (strip - 3) * SZ
                        c1 = (strip + 1) * SZ
                        nc.scalar.dma_start(
                            out=o_d[:, c0:c1], in_=ot[:, c0:c1]
                        )

    nc.compile()
    return nc


def get_nc():
    if "nc" not in _NC_CACHE:
        _NC_CACHE["nc"] = _build_nc()
    return _NC_CACHE["nc"]


def prep_inputs(x, weight, bias):
    """Host-side shard + layout prep. Returns per-core in_maps."""
    x = np.asarray(x, dtype=np.float32)
    weight = np.asarray(weight, dtype=np.float32)
    bias = np.asarray(bias, dtype=np.float32)

    # w_prep[oh, j=kj*32+c, q=ki, ow, o]; j=96 row: 0 for q<2, bias for q=2
    wp = np.zeros((N_CORES * ROWS, KP, 3, OW, C_OUT), NP_BF16)
    wp[:OH, :96] = (
        weight.transpose(1, 5, 3, 4, 2, 0).reshape(OH, 96, 3, OW, C_OUT)
    ).astype(NP_BF16)
    wp[:OH, 96, 2] = bias.transpose(1, 2, 0).astype(NP_BF16)
    # split ow into half-row strips + pad each line: [row, half, j, (q l o)+32]
    wp = wp.reshape(N_CORES * ROWS, KP, 3, 2, HALF, C_OUT).transpose(0, 3, 1, 2, 4, 5)
    wline = 3 * HALF * C_OUT + 32
    wpad = np.zeros((N_CORES * ROWS, 2, KP, wline), NP_BF16)
    wpad[:, :, :, : 3 * HALF * C_OUT] = wp.reshape(
        N_CORES * ROWS, 2, KP, 3 * HALF * C_OUT
    )
    wp = wpad

    xp = np.zeros((B, C_IN, N_CORES * ROWS + 2, W), NP_BF16)
    xp[:, :, :H] = x.astype(NP_BF16)
    xt = xp.transpose(1, 2, 3, 0)  # [c, h, w, b]

    ones = np.ones((1, XH * OW * B), NP_BF16)

    in_maps = []
    for c in range(N_CORES):
        r0 = c * ROWS
        xc = xt[:, r0 : r0 + XH]  # [c, 10, 64, b]
        xsh = np.stack([xc[:, :, kj : kj + OW, :] for kj in range(KK)])
        in_maps.append(
            {
                "x": np.ascontiguousarray(xsh),
                "w": np.ascontiguousarray(wp[r0 : r0 + ROWS]),
                "ones": ones,
            }
        )
    return in_maps


def gather_output(results):
    """results: list of per-core out dicts -> full [B, C_OUT, OH, OW] fp32."""
    out = np.empty((B, C_OUT, OH, OW), np.float32)
    for c in range(N_CORES):
        # out[p=(l4,b), (row, half, grp, o)]
        oc = np.asarray(results[c]["out"]).astype(np.float32)
        v = oc.reshape(4, B, ROWS, 2, NG, C_OUT)
        # ow = half*31 + grp*4 + l  (grp*4+l < 31)
        arr = v.transpose(1, 5, 2, 3, 4, 0).reshape(B, C_OUT, ROWS, 2, 32)
        arr = arr[:, :, :, :, :HALF].reshape(B, C_OUT, ROWS, OW)
        r0 = c * ROWS
        rows = min(ROWS, OH - r0)
        out[:, :, r0 : r0 + rows, :] = arr[:, :, :rows, :]
    return out


def run(inputs, **kw):
    nc = get_nc()
    in_maps = prep_inputs(inputs["x"], inputs["weight"], inputs["bias"])
    res = run_bass_kernel_spmd(nc, in_maps, core_ids=list(range(N_CORES)), **kw)
    return gather_output(res.results), res


def kernel(x, weight, bias):
    out, _ = run({"x": x, "weight": weight, "bias": bias})
    return out


# revision 13
# speedup vs baseline: 5.7089x; 5.7089x over previous
"""LocallyConnected2d Trainium2 kernel (bf16 pipeline).

Problem: out[b,o,oh,ow] = sum_{c,ki,kj} x[b,c,oh+ki,ow+kj] * W[o,oh,ow,c,ki,kj] + bias[o,oh,ow]
Shapes: x[32,32,64,64], W[64,62,62,32,3,3], bias[64,62,62] -> out[32,64,62,62], fp32 I/O.

The untied weight tensor (283 MB fp32) is read exactly once -> the kernel is
HBM-bandwidth bound. All operands ship as bf16 (accuracy gate 2e-2 vs ~2e-3
bf16 quantization error), halving the dominant weight stream; PSUM accumulates
in fp32; the output returns as bf16 and is upcast on host.

Strategy (8 NeuronCores, sharded over output rows, 8 rows/core padded to 64):
- Per output location: 3 accumulating PE matmuls, K=97 each (chunk q = kernel
  row ki; features j=(kj,c) plus a ones-row at j=96 that carries bias on q=2).
- lhsT (stationary) = x patch columns [97,32b]: x is loaded into SBUF once as
  3 column-shifted replicas on partitions kj*32+c, so every lhsT is a direct
  AP slice (no im2col data movement). Partition 96 = constant 1.0.
- rhs (moving) = per-location weights [97,64o], streamed from HBM in
  half-row strips with a host-side layout [row, half, j, q, ow, o] making each
  strip one fully-contiguous DMA (97 x 11.9KB descriptors).
- One PSUM bank [128,512] per strip accumulates 8 location-groups (4 locations
  x 32b on partitions, 64o per group in free); a single DVE copy casts the
  bank to a bf16 SBUF strip; one contiguous DMA per half-row out.
"""

import numpy as np
import ml_dtypes

import concourse.bass as bass  # noqa: F401
import concourse.mybir as mybir
import concourse.tile as tile
from concourse import bacc
from concourse.bass_utils import run_bass_kernel_spmd

B, C_IN, H, W = 32, 32, 64, 64
C_OUT, OH, OW, KK = 64, 62, 62, 3
N_CORES = 8
ROWS = 8          # padded output rows per core (8*8=64 >= 62)
HALF = 31         # locations per strip (half an output row)
XH = ROWS + 2     # input rows needed per core
KP = 97           # contraction per chunk: 96 features + ones/bias row
NG = 8            # ceil(31/4) location groups per strip
F32 = mybir.dt.float32
BF16 = mybir.dt.bfloat16
NP_BF16 = ml_dtypes.bfloat16

_NC_CACHE = {}


def _build_nc():
    nc = bacc.Bacc(
        "TRN2",
        target_bir_lowering=False,
        debug=False,
        enable_asserts=False,
        num_devices=N_CORES,
    )
    # x ships host-transposed AND pre-shifted into 3 kj-replicas
    # [kj, c, h, w(62), b] so the whole x3 load is one contiguous DMA
    x_d = nc.dram_tensor("x", [KK, C_IN, XH, OW, B], BF16, kind="ExternalInput").ap()
    # w ships pre-split by half-row strip: [row, half, j, (q l o)+pad]. The
    # 32-elem line pad makes the DRAM source non-contiguous across
    # partitions: a fully-contiguous source lets the HWDGE M2S-concat fuse
    # all descriptors into ONE SDMA engine's stream (~27 GB/s measured);
    # with stride != length the 97 line-descriptors spread over all 16
    # engines (~16x).
    WLINE = 3 * HALF * C_OUT + 32  # 5984
    w_d = nc.dram_tensor(
        "w", [ROWS, 2, KP, WLINE], BF16, kind="ExternalInput"
    ).ap()
    ones_d = nc.dram_tensor("ones", [1, XH * OW * B], BF16, kind="ExternalInput").ap()
    # out layout: [p=(l4,b), strip, grp, o] - partition-major so each store
    # DMA covers several strips with fat contiguous per-partition lines;
    # host unscrambles + upcasts
    o_d = nc.dram_tensor(
        "out", [128, ROWS * 2 * NG * C_OUT], BF16, kind="ExternalOutput"
    ).ap()

    with tile.TileContext(nc) as tc:
        with (
            tc.tile_pool(name="xpool", bufs=1) as xpool,
            tc.tile_pool(name="wpool", bufs=6) as wpool,
            tc.tile_pool(name="opool", bufs=1) as opool,
            tc.tile_pool(name="pspool", bufs=3, space="PSUM") as pspool,
        ):
            # x replicas: partition kj*32+c holds x[b,c,h,w+kj] at free
            # (h, w, b); partition 96 = 1.0 (carries the bias row).
            # SWDGE (gpsimd) sprays each partition line into 16 tiny
            # descriptors (~97 GB/s measured); everything rides the two
            # HWDGE rings (sync=weights, scalar=x tail + out) instead.
            HZ = OW * B  # 1984
            x3 = xpool.tile([KP, XH * HZ], BF16)
            xsrc = x_d.rearrange("k c h w b -> (k c) (h w b)")

            def load_x_rows(r0, r1, eng):
                eng.dma_start(
                    out=x3[0:96, r0 * HZ : r1 * HZ],
                    in_=xsrc[0:96, r0 * HZ : r1 * HZ],
                )

            # rows 0-2 feed strip 0; they go ahead of the w strips on the
            # sync ring. Rows 3-9 + ones ride the scalar ring concurrently.
            load_x_rows(0, 3, nc.sync)
            nc.scalar.dma_start(out=x3[96:97, :], in_=ones_d)
            load_x_rows(3, 6, nc.scalar)
            load_x_rows(6, 10, nc.scalar)

            QZ = HALF * C_OUT  # 1984, one chunk per kernel row q
            ot = opool.tile([128, ROWS * 2 * NG * C_OUT], BF16)  # all strips
            for row in range(ROWS):
                for half in range(2):
                    strip = row * 2 + half
                    # HWDGE fans a DMA across the 16 SDMA engines only when
                    # the outer (line) dim is divisible by 16 — a 97-line
                    # transfer lands on ONE engine (~27 GB/s). Split each
                    # strip into the 96 feature lines (spreads 16-way) and
                    # the single bias line.
                    wt_full = wpool.tile([KP, 3 * QZ + 32], BF16, tag="wt")
                    wt = wt_full[:, 0 : 3 * QZ]
                    wsrc = w_d[row, half][:, 0 : 3 * QZ]
                    nc.scalar.dma_start(out=wt[96:97, :], in_=wsrc[96:97, :])
                    if strip in (0, 15):
                        # split by q-chunk so the first q=0 matmuls of the
                        # first/last strip unblock after 1/3 of the strip
                        for f0, f1 in ((0, QZ), (QZ, 2 * QZ), (2 * QZ, 3 * QZ)):
                            nc.sync.dma_start(
                                out=wt[0:96, f0:f1], in_=wsrc[0:96, f0:f1]
                            )
                    else:
                        nc.sync.dma_start(out=wt[0:96, :], in_=wsrc[0:96, :])
                    # one PSUM bank for the whole strip: partitions (l4,b),
                    # free (grp, o)
                    ps = pspool.tile([128, NG * C_OUT], F32, tag="ps")
                    for g in range(NG):
                        gn = min(4, HALF - g * 4)  # 4,4,...,3
                        for li in range(4):
                            # pad slot in the last group duplicates the prior
                            # location (keeps PSUM fully written; host drops it)
                            eff = min(li, gn - 1)
                            ow = half * HALF + g * 4 + eff
                            loff = (g * 4 + eff) * C_OUT
                            for q in range(3):
                                nc.tensor.matmul(
                                    ps[32 * li : 32 * li + 32, g * C_OUT : (g + 1) * C_OUT],
                                    x3[
                                        :,
                                        (row + q) * HZ
                                        + ow * B : (row + q) * HZ
                                        + ow * B
                                        + B,
                                    ],  # [97, 32] lhsT
                                    wt[:, q * QZ + loff : q * QZ + loff + C_OUT],
                                    start=(q == 0),
                                    stop=(q == 2),
                                    tile_position=(0, 32 * li),
                                )
                    SZ = NG * C_OUT  # 512 out elems per strip per partition
                    nc.vector.tensor_copy(
                        out=ot[:, strip * SZ : (strip + 1) * SZ], in_=ps
                    )
                    # store in 4-strip chunks: 4KB contiguous per-partition
                    # lines on the scalar HWDGE ring
                    if strip % 4 == 3:
                        c0 = (strip - 3) * SZ
                        c1 = (strip + 1) * SZ
                        nc.scalar.dma_start(
                            out=o_d[:, c0:c1], in_=ot[:, c0:c1]
                        )

    nc.compile()
    return nc


def get_nc():
    if "nc" not in _NC_CACHE:
        _NC_CACHE["nc"] = _build_nc()
    return _NC_CACHE["nc"]


def prep_inputs(x, weight, bias):
    """Host-side shard + layout prep. Returns per-core in_maps."""
    x = np.asarray(x, dtype=np.float32)
    weight = np.asarray(weight, dtype=np.float32)
    bias = np.asarray(bias, dtype=np.float32)

    # w_prep[oh, j=kj*32+c, q=ki, ow, o]; j=96 row: 0 for q<2, bias for q=2
    wp = np.zeros((N_CORES * ROWS, KP, 3, OW, C_OUT), NP_BF16)
    wp[:OH, :96] = (
        weight.transpose(1, 5, 3, 4, 2, 0).reshape(OH, 96, 3, OW, C_OUT)
    ).astype(NP_BF16)
    wp[:OH, 96, 2] = bias.transpose(1, 2, 0).astype(NP_BF16)
    # split ow into half-row strips + pad each line: [row, half, j, (q l o)+32]
    wp = wp.reshape(N_CORES * ROWS, KP, 3, 2, HALF, C_OUT).transpose(0, 3, 1, 2, 4, 5)
    wline = 3 * HALF * C_OUT + 32
    wpad = np.zeros((N_CORES * ROWS, 2, KP, wline), NP_BF16)
    wpad[:, :, :, : 3 * HALF * C_OUT] = wp.reshape(
        N_CORES * ROWS, 2, KP, 3 * HALF * C_OUT
    )
    wp = wpad

    xp = np.zeros((B, C_IN, N_CORES * ROWS + 2, W), NP_BF16)
    xp[:, :, :H] = x.astype(NP_BF16)
    xt = xp.transpose(1, 2, 3, 0)  # [c, h, w, b]

    ones = np.ones((1, XH * OW * B), NP_BF16)

    in_maps = []
    for c in range(N_CORES):
        r0 = c * ROWS
        xc = xt[:, r0 : r0 + XH]  # [c, 10, 64, b]
        xsh = np.stack([xc[:, :, kj : kj + OW, :] for kj in range(KK)])
        in_maps.append(
            {
                "x": np.ascontiguousarray(xsh),
                "w": np.ascontiguousarray(wp[r0 : r0 + ROWS]),
                "ones": ones,
            }
        )
    return in_maps


def gather_output(results):
    """results: list of per-core out dicts -> full [B, C_OUT, OH, OW] fp32."""
    out = np.empty((B, C_OUT, OH, OW), np.float32)
    for c in range(N_CORES):
        # out[p=(l4,b), (row, half, grp, o)]
        oc = np.asarray(results[c]["out"]).astype(np.float32)
        v = oc.reshape(4, B, ROWS, 2, NG, C_OUT)
        # ow = half*31 + grp*4 + l  (grp*4+l < 31)
        arr = v.transpose(1, 5, 2, 3, 4, 0).reshape(B, C_OUT, ROWS, 2, 32)
        arr = arr[:, :, :, :, :HALF].reshape(B, C_OUT, ROWS, OW)
        r0 = c * ROWS
        rows = min(ROWS, OH - r0)
        out[:, :, r0 : r0 + rows, :] = arr[:, :, :rows, :]
    return out


def run(inputs, **kw):
    nc = get_nc()
    in_maps = prep_inputs(inputs["x"], inputs["weight"], inputs["bias"])
    res = run_bass_kernel_spmd(nc, in_maps, core_ids=list(range(N_CORES)), **kw)
    return gather_output(res.results), res


def kernel(x, weight, bias):
    out, _ = run({"x": x, "weight": weight, "bias": bias})
    return out


# revision 14
# speedup vs baseline: 5.7479x; 1.0068x over previous
"""LocallyConnected2d Trainium2 kernel (bf16 pipeline).

Problem: out[b,o,oh,ow] = sum_{c,ki,kj} x[b,c,oh+ki,ow+kj] * W[o,oh,ow,c,ki,kj] + bias[o,oh,ow]
Shapes: x[32,32,64,64], W[64,62,62,32,3,3], bias[64,62,62] -> out[32,64,62,62], fp32 I/O.

The untied weight tensor (283 MB fp32) is read exactly once -> the kernel is
HBM-bandwidth bound. All operands ship as bf16 (accuracy gate 2e-2 vs ~2e-3
bf16 quantization error), halving the dominant weight stream; PSUM accumulates
in fp32; the output returns as bf16 and is upcast on host.

Strategy (8 NeuronCores, sharded over output rows, 8 rows/core padded to 64):
- Per output location: 3 accumulating PE matmuls, K=97 each (chunk q = kernel
  row ki; features j=(kj,c) plus a ones-row at j=96 that carries bias on q=2).
- lhsT (stationary) = x patch columns [97,32b]: x is loaded into SBUF once as
  3 column-shifted replicas on partitions kj*32+c, so every lhsT is a direct
  AP slice (no im2col data movement). Partition 96 = constant 1.0.
- rhs (moving) = per-location weights [97,64o], streamed from HBM in
  half-row strips with a host-side layout [row, half, j, q, ow, o] making each
  strip one fully-contiguous DMA (97 x 11.9KB descriptors).
- One PSUM bank [128,512] per strip accumulates 8 location-groups (4 locations
  x 32b on partitions, 64o per group in free); a single DVE copy casts the
  bank to a bf16 SBUF strip; one contiguous DMA per half-row out.
"""

import numpy as np
import ml_dtypes

import concourse.bass as bass  # noqa: F401
import concourse.mybir as mybir
import concourse.tile as tile
from concourse import bacc
from concourse.bass_utils import run_bass_kernel_spmd

B, C_IN, H, W = 32, 32, 64, 64
C_OUT, OH, OW, KK = 64, 62, 62, 3
N_CORES = 8
ROWS = 8          # padded output rows per core (8*8=64 >= 62)
HALF = 31         # locations per strip (half an output row)
XH = ROWS + 2     # input rows needed per core
KP = 97           # contraction per chunk: 96 features + ones/bias row
NG = 8            # ceil(31/4) location groups per strip
F32 = mybir.dt.float32
BF16 = mybir.dt.bfloat16
NP_BF16 = ml_dtypes.bfloat16

_NC_CACHE = {}


def _build_nc():
    nc = bacc.Bacc(
        "TRN2",
        target_bir_lowering=False,
        debug=False,
        enable_asserts=False,
        num_devices=N_CORES,
    )
    # x ships host-transposed AND pre-shifted into 3 kj-replicas
    # [kj, c, h, w(62), b] so the whole x3 load is one contiguous DMA
    x_d = nc.dram_tensor("x", [KK, C_IN, XH, OW, B], BF16, kind="ExternalInput").ap()
    # w ships pre-split by half-row strip: [row, half, j, (q l o)+pad]. The
    # 32-elem line pad makes the DRAM source non-contiguous across
    # partitions: a fully-contiguous source lets the HWDGE M2S-concat fuse
    # all descriptors into ONE SDMA engine's stream (~27 GB/s measured);
    # with stride != length the 97 line-descriptors spread over all 16
    # engines (~16x).
    WLINE = 3 * HALF * C_OUT + 32  # 5984
    w_d = nc.dram_tensor(
        "w", [ROWS, 2, KP, WLINE], BF16, kind="ExternalInput"
    ).ap()
    ones_d = nc.dram_tensor("ones", [1, XH * OW * B], BF16, kind="ExternalInput").ap()
    # out layout: [p=(l4,b), strip, grp, o] - partition-major so each store
    # DMA covers several strips with fat contiguous per-partition lines;
    # host unscrambles + upcasts
    o_d = nc.dram_tensor(
        "out", [128, ROWS * 2 * NG * C_OUT], BF16, kind="ExternalOutput"
    ).ap()

    with tile.TileContext(nc) as tc:
        with (
            tc.tile_pool(name="xpool", bufs=1) as xpool,
            tc.tile_pool(name="wpool", bufs=6) as wpool,
            tc.tile_pool(name="opool", bufs=1) as opool,
            tc.tile_pool(name="pspool", bufs=3, space="PSUM") as pspool,
        ):
            # x replicas: partition kj*32+c holds x[b,c,h,w+kj] at free
            # (h, w, b); partition 96 = 1.0 (carries the bias row).
            # SWDGE (gpsimd) sprays each partition line into 16 tiny
            # descriptors (~97 GB/s measured); everything rides the two
            # HWDGE rings (sync=weights, scalar=x tail + out) instead.
            HZ = OW * B  # 1984
            x3 = xpool.tile([KP, XH * HZ], BF16)
            xsrc = x_d.rearrange("k c h w b -> (k c) (h w b)")

            def load_x_rows(r0, r1, eng):
                eng.dma_start(
                    out=x3[0:96, r0 * HZ : r1 * HZ],
                    in_=xsrc[0:96, r0 * HZ : r1 * HZ],
                )

            # rows 0-2 feed strip 0; they go ahead of the w strips on the
            # sync ring. Rows 3-9 + ones ride the scalar ring concurrently.
            load_x_rows(0, 3, nc.sync)
            nc.scalar.dma_start(out=x3[96:97, :], in_=ones_d)
            load_x_rows(3, 6, nc.scalar)
            load_x_rows(6, 10, nc.scalar)

            QZ = HALF * C_OUT  # 1984, one chunk per kernel row q
            ot = opool.tile([128, ROWS * 2 * NG * C_OUT], BF16)  # all strips
            for row in range(ROWS):
                for half in range(2):
                    strip = row * 2 + half
                    # HWDGE fans a DMA across the 16 SDMA engines only when
                    # the outer (line) dim is divisible by 16 — a 97-line
                    # transfer lands on ONE engine (~27 GB/s). Split each
                    # strip into the 96 feature lines (spreads 16-way) and
                    # the single bias line. Strips alternate between the two
                    # HWDGE rings so every SDMA engine sees packets from two
                    # queues and overlaps per-packet HBM latency.
                    weng = nc.sync if strip % 2 == 0 else nc.scalar
                    wt_full = wpool.tile([KP, 3 * QZ + 32], BF16, tag="wt")
                    wt = wt_full[:, 0 : 3 * QZ]
                    wsrc = w_d[row, half][:, 0 : 3 * QZ]
                    weng.dma_start(out=wt[96:97, :], in_=wsrc[96:97, :])
                    if strip in (0, 15):
                        # split by q-chunk so the first q=0 matmuls of the
                        # first/last strip unblock after 1/3 of the strip
                        for f0, f1 in ((0, QZ), (QZ, 2 * QZ), (2 * QZ, 3 * QZ)):
                            weng.dma_start(
                                out=wt[0:96, f0:f1], in_=wsrc[0:96, f0:f1]
                            )
                    else:
                        weng.dma_start(out=wt[0:96, :], in_=wsrc[0:96, :])
                    # one PSUM bank for the whole strip: partitions (l4,b),
                    # free (grp, o)
                    ps = pspool.tile([128, NG * C_OUT], F32, tag="ps")
                    for g in range(NG):
                        gn = min(4, HALF - g * 4)  # 4,4,...,3
                        # q-outer, li-inner: waves of 4 matmuls hit the 4 PE
                        # col-groups back-to-back so their rhs streams overlap
                        # in the array (li-outer would serialize on each
                        # col-group's accumulation chain)
                        for q in range(3):
                            for li in range(4):
                                # pad slot in the last group duplicates the
                                # prior location (keeps PSUM fully written;
                                # host drops it)
                                eff = min(li, gn - 1)
                                ow = half * HALF + g * 4 + eff
                                loff = (g * 4 + eff) * C_OUT
                                nc.tensor.matmul(
                                    ps[32 * li : 32 * li + 32, g * C_OUT : (g + 1) * C_OUT],
                                    x3[
                                        :,
                                        (row + q) * HZ
                                        + ow * B : (row + q) * HZ
                                        + ow * B
                                        + B,
                                    ],  # [97, 32] lhsT
                                    wt[:, q * QZ + loff : q * QZ + loff + C_OUT],
                                    start=(q == 0),
                                    stop=(q == 2),
                                    tile_position=(0, 32 * li),
                                )
                    SZ = NG * C_OUT  # 512 out elems per strip per partition
                    nc.vector.tensor_copy(
                        out=ot[:, strip * SZ : (strip + 1) * SZ], in_=ps
                    )
                    # store in 4-strip chunks: 4KB contiguous per-partition
                    # lines on the scalar HWDGE ring
                    if strip % 4 == 3:
                        c0 = (strip - 3) * SZ
                        c1 = (strip + 1) * SZ
                        nc.scalar.dma_start(
                            out=o_d[:, c0:c1], in_=ot[:, c0:c1]
                        )

    nc.compile()
    return nc


def get_nc():
    if "nc" not in _NC_CACHE:
        _NC_CACHE["nc"] = _build_nc()
    return _NC_CACHE["nc"]


def prep_inputs(x, weight, bias):
    """Host-side shard + layout prep. Returns per-core in_maps."""
    x = np.asarray(x, dtype=np.float32)
    weight = np.asarray(weight, dtype=np.float32)
    bias = np.asarray(bias, dtype=np.float32)

    # w_prep[oh, j=kj*32+c, q=ki, ow, o]; j=96 row: 0 for q<2, bias for q=2
    wp = np.zeros((N_CORES * ROWS, KP, 3, OW, C_OUT), NP_BF16)
    wp[:OH, :96] = (
        weight.transpose(1, 5, 3, 4, 2, 0).reshape(OH, 96, 3, OW, C_OUT)
    ).astype(NP_BF16)
    wp[:OH, 96, 2] = bias.transpose(1, 2, 0).astype(NP_BF16)
    # split ow into half-row strips + pad each line: [row, half, j, (q l o)+32]
    wp = wp.reshape(N_CORES * ROWS, KP, 3, 2, HALF, C_OUT).transpose(0, 3, 1, 2, 4, 5)
    wline = 3 * HALF * C_OUT + 32
    wpad = np.zeros((N_CORES * ROWS, 2, KP, wline), NP_BF16)
    wpad[:, :, :, : 3 * HALF * C_OUT] = wp.reshape(
        N_CORES * ROWS, 2, KP, 3 * HALF * C_OUT
    )
    wp = wpad

    xp = np.zeros((B, C_IN, N_CORES * ROWS + 2, W), NP_BF16)
    xp[:, :, :H] = x.astype(NP_BF16)
    xt = xp.transpose(1, 2, 3, 0)  # [c, h, w, b]

    ones = np.ones((1, XH * OW * B), NP_BF16)

    in_maps = []
    for c in range(N_CORES):
        r0 = c * ROWS
        xc = xt[:, r0 : r0 + XH]  # [c, 10, 64, b]
        xsh = np.stack([xc[:, :, kj : kj + OW, :] for kj in range(KK)])
        in_maps.append(
            {
                "x": np.ascontiguousarray(xsh),
                "w": np.ascontiguousarray(wp[r0 : r0 + ROWS]),
                "ones": ones,
            }
        )
    return in_maps


def gather_output(results):
    """results: list of per-core out dicts -> full [B, C_OUT, OH, OW] fp32."""
    out = np.empty((B, C_OUT, OH, OW), np.float32)
    for c in range(N_CORES):
        # out[p=(l4,b), (row, half, grp, o)]
        oc = np.asarray(results[c]["out"]).astype(np.float32)
        v = oc.reshape(4, B, ROWS, 2, NG, C_OUT)
        # ow = half*31 + grp*4 + l  (grp*4+l < 31)
        arr = v.transpose(1, 5, 2, 3, 4, 0).reshape(B, C_OUT, ROWS, 2, 32)
        arr = arr[:, :, :, :, :HALF].reshape(B, C_OUT, ROWS, OW)
        r0 = c * ROWS
        rows = min(ROWS, OH - r0)
        out[:, :, r0 : r0 + rows, :] = arr[:, :, :rows, :]
    return out


def run(inputs, **kw):
    nc = get_nc()
    in_maps = prep_inputs(inputs["x"], inputs["weight"], inputs["bias"])
    res = run_bass_kernel_spmd(nc, in_maps, core_ids=list(range(N_CORES)), **kw)
    return gather_output(res.results), res


def kernel(x, weight, bias):
    out, _ = run({"x": x, "weight": weight, "bias": bias})
    return out
